# revision 10
# baseline (speedup 1.0000x reference)
"""CenterLoss Trainium2 kernel (fp8 DoubleRow streaming version).

Full inputs:
  ep_mask_embed    (8, 4096, 256) f32
  ep_mask          (8, 1, 1024, 1024) f32
  query_mask_embed (8, 4096, 256) f32
  query_mask       (8, 1, 1024, 1024) f32
Output: (3,) f32 = [mean(center_loss), mean(pos_loss), mean(neg_loss)]

Sharding: data-parallel, one batch sample per NeuronCore (8 cores).

The loss expands into per-sample sufficient statistics: epw = [m;1-m]^T ep
and qw = [m;1-m]^T q (channel sums, PE DoubleRow matmul chains), the
scalar scol = [m;1-m]^T rowsum(q^2) (DVE reduce + tiny f32 matmul), and
the mask counts (host, from the downsample it already does).  The ~50
scalar flops downstream per sample happen on host where the batch mean
over the 8 per-core results already lives.

Built around the memory roofline (358 GB/s/core):
  - Embeds ship fp8 e4m3 (q^2 precomputed on host, fp8): 3 MB/core vs
    8 MB f32.  Measured fp8 rel-err ~7e-4 on the loss; tolerance 2e-2.
  - ep/q matmuls run DoubleRow (2 tokens/partition/cycle): 32 pieces of
    [128,2,256] x [128,2,2] at ~183ns each; the dual-row weight AP dim
    needs step%16==0, hence the two 64-col ks-planes in lw.
  - The q^2 chain rides the otherwise-idle Vector engine instead of the
    PE: rowsum r_t, then mask-dot via one f32 matmul against ones.
  - Tokens stage as [128, 16*256] fp8 -> 4KB contiguous descriptor per
    partition (the size at which the DMA queues sustain full BW).
  - Every 512KB stream is split into partition-halves issued on BOTH
    HWDGE queues (sync + scalar) so all 16 DMA engines engage from the
    start and no single transfer straggles at the end.
  - The three output sections DMA out independently the moment each is
    final, so only the last (qw) issue sits on the tail.
"""

import numpy as np
import ml_dtypes
from contextlib import ExitStack

import concourse.bass as bass
import concourse.bacc as bacc
import concourse.tile as tile
from concourse import mybir
from concourse.bass_utils import run_bass_kernel_spmd

F32 = mybir.dt.float32
F8 = mybir.dt.float8e4
NP_F8 = ml_dtypes.float8_e4m3fn

P = 128          # partitions
HP = P // 2      # half-partition split per DMA queue
N_TOK = 4096     # tokens per sample (64*64 patches)
C = 256          # channels
T = 16           # tokens per partition per chunk (4KB fp8 descriptor)
DC = P * T       # tokens per chunk (2048)
N_DC = N_TOK // DC   # 2 chunks
NPC = T // 2     # parity-pairs (pieces) per chunk: 8
B = 8            # batch == n cores
PATCH = 16
OUTW = 516       # epw 256 | qw 256 | scol 1 | pad

_CACHE = {}


def _build():
    """Build the per-core Bass program (identical on all cores)."""
    nc = bacc.Bacc("TRN2", target_bir_lowering=False, debug=False)

    ep8 = nc.dram_tensor("ep8", [N_TOK, C], F8, kind="ExternalInput").ap()
    q8 = nc.dram_tensor("q8", [N_TOK, C], F8, kind="ExternalInput").ap()
    qsq8 = nc.dram_tensor("qsq8", [N_TOK, C], F8, kind="ExternalInput").ap()
    # DoubleRow mask weights, two 64-col ks-planes (dual-fp8 ldweights
    # needs the ks dim step %16==0): col = 64*ks + 4*jj + m,
    # m in (q_pos, q_neg, ep_pos, ep_neg),
    # token = 2048*(jj//8) + 16*p + 2*(jj%8) + ks
    lw = nc.dram_tensor("lw", [P, 8 * N_DC * NPC], F8, kind="ExternalInput").ap()
    # q mask f32 for the DVE scol path: col = 16*i + g,
    # token = 2048*i + 16*p + g
    mq = nc.dram_tensor("mq", [P, N_DC * T], F32, kind="ExternalInput").ap()
    out = nc.dram_tensor("out", [2, OUTW], F32, kind="ExternalOutput").ap()

    DR = mybir.MatmulPerfMode.DoubleRow
    OP = mybir.AluOpType

    with tile.TileContext(nc) as tc, ExitStack() as ctx:
        const_pool = ctx.enter_context(tc.tile_pool(name="const", bufs=1))
        x_pool = ctx.enter_context(tc.tile_pool(name="x_pool", bufs=1))
        psum_pool = ctx.enter_context(
            tc.tile_pool(name="psum", bufs=1, space=bass.MemorySpace.PSUM)
        )
        fin_pool = ctx.enter_context(tc.tile_pool(name="fin", bufs=1))

        lw_t = const_pool.tile([P, 8 * N_DC * NPC], F8, name="lw_t", tag="lw_t")
        nc.scalar.dma_start(out=lw_t[:], in_=lw[:])
        mq_t = const_pool.tile([P, N_DC * T], F32, name="mq_t", tag="mq_t")
        nc.scalar.dma_start(out=mq_t[:], in_=mq[:])
        ones1 = const_pool.tile([P, 1], F32, name="ones1", tag="ones1")
        nc.vector.memset(ones1[:], 1.0)

        # streams in consumption order; each split into partition halves
        # issued on the two HWDGE queues so both advance every tile
        STREAMS = [("ep", 0), ("q", 0), ("qsq", 0), ("qsq", 1),
                   ("ep", 1), ("q", 1)]
        SRC = {"ep": ep8, "q": q8, "qsq": qsq8}
        X = {}
        for nm, i in STREAMS:
            X[(nm, i)] = x_pool.tile(
                [P, T * C], F8, name=f"x{nm}{i}", tag=f"x{nm}{i}")
        for half, eng in ((0, nc.sync), (1, nc.scalar)):
            for nm, i in STREAMS:
                t_ = X[(nm, i)]
                lo = half * HP
                rows = SRC[nm][i * DC + T * lo: i * DC + T * (lo + HP), :]
                eng.dma_start(
                    out=t_[lo:lo + HP, :],
                    in_=rows.rearrange("(p t) c -> p (t c)", t=T),
                )

        psum = {
            nm: psum_pool.tile([2, C], F32, name=f"ps_{nm}", tag=f"ps_{nm}")
            for nm in ("ep", "q")
        }
        psum_s = psum_pool.tile([2, 1], F32, name="ps_s", tag="ps_s")
        fin = fin_pool.tile([2, OUTW], F32, name="fin", tag="fin")

        def mm_block(nm, i):
            for j in range(NPC):
                jj = NPC * i + j
                off = 4 * jj + (2 if nm == "ep" else 0)
                w = lw_t[:].rearrange(
                    "p (k c) -> p k c", k=2)[:, :, off:off + 2]
                rhs = X[(nm, i)][:, 512 * j:512 * (j + 1)].rearrange(
                    "p (k c) -> p k c", k=2)
                nc.tensor.matmul(
                    psum[nm][:], w, rhs,
                    start=(i == 0 and j == 0),
                    stop=(i == N_DC - 1 and j == NPC - 1),
                    perf_mode=DR,
                )

        # DVE scol path: r[p, 16i+g] = sum_c qsq[token(i,p,g), c]
        r = fin_pool.tile([P, N_DC * T], F32, name="r", tag="r")
        rp = fin_pool.tile([P, N_DC * T], F32, name="rp", tag="rp")
        s2 = fin_pool.tile([P, 2], F32, name="s2", tag="s2")

        # chunk 0: PE on ep/q, DVE on qsq
        mm_block("ep", 0)
        mm_block("q", 0)
        nc.vector.tensor_reduce(
            r[:, 0:T],
            X[("qsq", 0)][:].rearrange("p (g c) -> p g c", g=T),
            axis=mybir.AxisListType.X, op=OP.add,
        )
        # chunk 1 (DMA order qsq1 -> ep1 -> q1)
        nc.vector.tensor_reduce(
            r[:, T:2 * T],
            X[("qsq", 1)][:].rearrange("p (g c) -> p g c", g=T),
            axis=mybir.AxisListType.X, op=OP.add,
        )
        nc.vector.tensor_mul(rp[:], r[:], mq_t[:])
        nc.vector.tensor_reduce(
            s2[:, 0:1], rp[:], axis=mybir.AxisListType.X, op=OP.add)
        nc.vector.tensor_reduce(
            s2[:, 1:2], r[:], axis=mybir.AxisListType.X, op=OP.add)
        nc.vector.tensor_sub(s2[:, 1:2], s2[:, 1:2], s2[:, 0:1])

        mm_block("ep", 1)
        # ep chain complete: ship epw while q1 still streams/computes
        nc.vector.tensor_copy(fin[:, 0:C], psum["ep"][:])
        nc.scalar.dma_start(out=out[:, 0:C], in_=fin[:, 0:C])

        # scol partition-sum on PE (f32), shipped during q1 block
        nc.tensor.matmul(psum_s[:], s2[:], ones1[:])
        nc.vector.tensor_copy(fin[:, 2 * C:2 * C + 1], psum_s[:])
        nc.scalar.dma_start(
            out=out[:, 2 * C:2 * C + 1], in_=fin[:, 2 * C:2 * C + 1])

        mm_block("q", 1)
        nc.vector.tensor_copy(fin[:, C:2 * C], psum["q"][:])
        nc.scalar.dma_start(out=out[:, C:2 * C], in_=fin[:, C:2 * C])

    nc.compile()
    return nc


def get_nc():
    if "nc" not in _CACHE:
        _CACHE["nc"] = _build()
    return _CACHE["nc"]


# token index per (partition, piece jj, ks): DoubleRow weight layout
_PG = np.arange(P)[:, None, None]
_JJ = np.arange(N_DC * NPC)[None, :, None]
_KS = np.arange(2)[None, None, :]
_TOK = (DC * (_JJ // NPC) + T * _PG + 2 * (_JJ % NPC) + _KS)  # [128, 16, 2]
# token index per (partition, 16i+g) for the f32 q-mask tile
_TOKG = (DC * (np.arange(N_DC * T)[None, :] // T) + T * np.arange(P)[:, None]
         + (np.arange(N_DC * T)[None, :] % T))  # [128, 32]


def _mask_ds(mask_b):
    """Downsample one sample's mask (nearest, stride 16) -> (4096,) f64."""
    return mask_b[0, ::PATCH, ::PATCH].reshape(-1).astype(np.float64)


def make_in_maps(ep_mask_embed, ep_mask, query_mask_embed, query_mask):
    in_maps, counts = [], []
    for b in range(B):
        em = _mask_ds(ep_mask[b])
        qm = _mask_ds(query_mask[b])
        et = em[_TOK]  # [128, 16, 2] = (p, jj, ks)
        qt = qm[_TOK]
        L = np.stack([qt, 1.0 - qt, et, 1.0 - et], axis=-1)  # [p,jj,ks,m]
        lw_b = L.transpose(0, 2, 1, 3)  # [p, ks, jj, m] -> col 64ks+4jj+m
        in_maps.append({
            "ep8": np.ascontiguousarray(ep_mask_embed[b]).astype(NP_F8),
            "q8": np.ascontiguousarray(query_mask_embed[b]).astype(NP_F8),
            "qsq8": np.square(query_mask_embed[b]).astype(NP_F8),
            "lw": np.ascontiguousarray(
                lw_b.reshape(P, 8 * N_DC * NPC)).astype(NP_F8),
            "mq": np.ascontiguousarray(qm[_TOKG]).astype(np.float32),
        })
        counts.append((em.sum(), (1.0 - em).sum(), qm.sum(), (1.0 - qm).sum()))
    return in_maps, counts


def finalize(per_core, counts):
    """per_core: list of 8 arrays [2, 516] (epw|qw|scol) -> full (3,)."""
    pos = np.zeros(B)
    neg = np.zeros(B)
    for b in range(B):
        st = np.asarray(per_core[b]).astype(np.float64)
        n_pe, n_ne, n_pq, n_nq = counts[b]
        epw, qw = st[:, 0:C], st[:, C:2 * C]
        scol = st[:, 2 * C]  # [pos, neg]
        pc = epw[0] / (n_pe + 0.1)
        ncen = epw[1] / (n_ne + 0.1)
        pn = scol[0] - 2.0 * (pc @ qw[0]) + n_pq * (pc @ pc)
        nn = scol[1] - 2.0 * (ncen @ qw[1]) + n_nq * (ncen @ ncen)
        pos[b] = pn / (max(n_pq, 1.0) * C) if n_pq > 0 else 0.0
        neg[b] = nn / (max(n_nq, 1.0) * C) if n_nq > 0 else 0.0
    return np.array(
        [(pos + neg).mean(), pos.mean(), neg.mean()], dtype=np.float32
    )


def kernel(ep_mask_embed, ep_mask, query_mask_embed, query_mask):
    ep_mask_embed = np.asarray(ep_mask_embed, dtype=np.float32)
    ep_mask = np.asarray(ep_mask, dtype=np.float32)
    query_mask_embed = np.asarray(query_mask_embed, dtype=np.float32)
    query_mask = np.asarray(query_mask, dtype=np.float32)

    nc = get_nc()
    in_maps, counts = make_in_maps(
        ep_mask_embed, ep_mask, query_mask_embed, query_mask)
    res = run_bass_kernel_spmd(nc, in_maps, list(range(B)))
    return finalize([r["out"] for r in res.results], counts)


# revision 13
# speedup vs baseline: 1.3563x; 1.3563x over previous
"""CenterLoss Trainium2 kernel (fp8 DoubleRow streaming version).

Full inputs:
  ep_mask_embed    (8, 4096, 256) f32
  ep_mask          (8, 1, 1024, 1024) f32
  query_mask_embed (8, 4096, 256) f32
  query_mask       (8, 1, 1024, 1024) f32
Output: (3,) f32 = [mean(center_loss), mean(pos_loss), mean(neg_loss)]

Sharding: data-parallel, one batch sample per NeuronCore (8 cores).

The loss expands into mask-weighted channel sums (see previous f32
version): per sample it needs epw = [m;1-m]^T ep, qw = [m;1-m]^T q,
qsqw = [m;1-m]^T q^2, plus the four mask counts.  All three are
PSUM-accumulated matmul chains; everything downstream is ~50 scalar
flops per sample done on host from those statistics (the same place the
batch mean over the 8 per-core results already happens).

This version is built around the memory roofline (358 GB/s/core):
  - Embeds ship as fp8 e4m3 (q^2 precomputed on host, also fp8): 3 MB
    per core instead of 8 MB f32.  Rel-err budget: fp8 rounding is
    ~0.07% on the final loss (measured); tolerance is 2e-2.
  - Matmuls run in DoubleRow perf mode: lhsT [128,2,M] fp8 contracts
    256 tokens per instruction at 2 rhs bytes/partition/cycle, so the
    3 chains stream well under the DMA time.
  - Tokens stage as [128, 16*256] fp8 -> one 4KB contiguous descriptor
    per partition (the size at which the DMA queues sustain full BW).
  - All six 512KB streams issue on the sync-engine HWDGE queue in
    consumption order (each DIRECT2D issue costs ~650ns serial, so
    fewer+bigger is better); the tiny weight/out DMAs ride the
    Activation-engine queue so they never stall the stream.
  - Mask weights (m, 1-m for ep and q, fp8, DoubleRow layout) are
    host-packed into one [128,128] tile; counts come from the host-side
    mask downsample it already does.
"""

import numpy as np
import ml_dtypes
from contextlib import ExitStack

import concourse.bass as bass
import concourse.bacc as bacc
import concourse.tile as tile
from concourse import mybir
from concourse.bass_utils import run_bass_kernel_spmd

F32 = mybir.dt.float32
F8 = mybir.dt.float8e4
NP_F8 = ml_dtypes.float8_e4m3fn

P = 128          # partitions
N_TOK = 4096     # tokens per sample (64*64 patches)
C = 256          # channels
T = 16           # tokens per partition per chunk (4KB fp8 descriptor)
DC = P * T       # tokens per chunk (2048)
N_DC = N_TOK // DC   # 2 chunks
NPC = T // 2     # parity-pairs (pieces) per chunk: 8
B = 8            # batch == n cores
PATCH = 16

_CACHE = {}


def _build():
    """Build the per-core Bass program (identical on all cores)."""
    nc = bacc.Bacc("TRN2", target_bir_lowering=False, debug=False)

    ep8 = nc.dram_tensor("ep8", [N_TOK, C], F8, kind="ExternalInput").ap()
    q8 = nc.dram_tensor("q8", [N_TOK, C], F8, kind="ExternalInput").ap()
    qsq8 = nc.dram_tensor("qsq8", [N_TOK, C], F8, kind="ExternalInput").ap()
    # host-packed DoubleRow mask weights.  The dual-fp8 ldweights ISA
    # check needs the dual-row AP dim to have num_elem==2 and a step
    # that is a multiple of 16 elements, so the two ks sub-rows live in
    # separate 64-col planes: col = 64*ks + 4*jj + m,
    # m in (q_pos, q_neg, ep_pos, ep_neg),
    # token = 2048*(jj//8) + 16*p + 2*(jj%8) + ks
    lw = nc.dram_tensor("lw", [P, 8 * N_DC * NPC], F8, kind="ExternalInput").ap()
    # [epw | qw | qsqw], rows = (pos, neg)
    out = nc.dram_tensor("out", [2, 3 * C], F32, kind="ExternalOutput").ap()

    DR = mybir.MatmulPerfMode.DoubleRow

    with tile.TileContext(nc) as tc, ExitStack() as ctx:
        const_pool = ctx.enter_context(tc.tile_pool(name="const", bufs=1))
        x_pool = ctx.enter_context(tc.tile_pool(name="x_pool", bufs=1))
        psum_pool = ctx.enter_context(
            tc.tile_pool(name="psum", bufs=1, space=bass.MemorySpace.PSUM)
        )
        fin_pool = ctx.enter_context(tc.tile_pool(name="fin", bufs=1))

        lw_t = const_pool.tile([P, 8 * N_DC * NPC], F8, name="lw_t", tag="lw_t")
        nc.sync.dma_start(out=lw_t[:], in_=lw[:])

        # six 512KB streams on the scalar (ACT) HWDGE queue in
        # consumption order — its sequencer clears the preamble ~2.5us
        # before sync's, so the stream starts that much earlier; keeping
        # all six sequential on ONE queue preserves the DRAM row
        # locality the DMA engines need for full bandwidth
        X = {}
        for i in range(N_DC):
            for nm, src in (("ep", ep8), ("q", q8), ("qsq", qsq8)):
                t_ = x_pool.tile([P, T * C], F8, name=f"x{nm}{i}", tag=f"x{nm}{i}")
                nc.scalar.dma_start(
                    out=t_[:],
                    in_=src[i * DC:(i + 1) * DC, :].rearrange(
                        "(p t) c -> p (t c)", t=T),
                )
                X[(nm, i)] = t_

        psum = {
            nm: psum_pool.tile([2, C], F32, name=f"ps_{nm}", tag=f"ps_{nm}")
            for nm in ("ep", "q", "qsq")
        }

        fin = fin_pool.tile([2, 3 * C], F32, name="fin", tag="fin")
        SEC = {"ep": 0, "q": 1, "qsq": 2}

        # chain-major matmul order so the PE stream never blocks on a
        # later DMA: all pieces of (chain, chunk) as soon as that
        # stream lands.  After a chain's stop-matmul its section ships
        # immediately (copy on idle DVE + out-DMA issue on idle sync
        # queue), so only the last chain's shipment sits on the tail.
        for i in range(N_DC):
            for nm in ("ep", "q", "qsq"):
                for j in range(NPC):
                    jj = NPC * i + j
                    off = 4 * jj + (2 if nm == "ep" else 0)
                    w = lw_t[:].rearrange(
                        "p (k c) -> p k c", k=2)[:, :, off:off + 2]
                    rhs = X[(nm, i)][:, 512 * j:512 * (j + 1)].rearrange(
                        "p (k c) -> p k c", k=2)
                    nc.tensor.matmul(
                        psum[nm][:], w, rhs,
                        start=(i == 0 and j == 0),
                        stop=(i == N_DC - 1 and j == NPC - 1),
                        perf_mode=DR,
                    )
                if i == N_DC - 1:
                    s = SEC[nm]
                    nc.vector.tensor_copy(
                        fin[:, s * C:(s + 1) * C], psum[nm][:])
                    nc.sync.dma_start(
                        out=out[:, s * C:(s + 1) * C],
                        in_=fin[:, s * C:(s + 1) * C])

    nc.compile()
    return nc


def get_nc():
    if "nc" not in _CACHE:
        _CACHE["nc"] = _build()
    return _CACHE["nc"]


# token index per (partition, piece jj, ks): DoubleRow weight layout
_PG = np.arange(P)[:, None, None]
_JJ = np.arange(N_DC * NPC)[None, :, None]
_KS = np.arange(2)[None, None, :]
_TOK = (DC * (_JJ // NPC) + T * _PG + 2 * (_JJ % NPC) + _KS)  # [128, 16, 2]


def _mask_ds(mask_b):
    """Downsample one sample's mask (nearest, stride 16) -> (4096,) f64."""
    return mask_b[0, ::PATCH, ::PATCH].reshape(-1).astype(np.float64)


def make_in_maps(ep_mask_embed, ep_mask, query_mask_embed, query_mask):
    in_maps, counts = [], []
    for b in range(B):
        em = _mask_ds(ep_mask[b])
        qm = _mask_ds(query_mask[b])
        et = em[_TOK]  # [128, 16, 2] = (p, jj, ks)
        qt = qm[_TOK]
        L = np.stack([qt, 1.0 - qt, et, 1.0 - et], axis=-1)  # [p,jj,ks,m]
        lw_b = L.transpose(0, 2, 1, 3)  # [p, ks, jj, m] -> col 64ks+4jj+m
        in_maps.append({
            "ep8": np.ascontiguousarray(ep_mask_embed[b]).astype(NP_F8),
            "q8": np.ascontiguousarray(query_mask_embed[b]).astype(NP_F8),
            "qsq8": np.square(query_mask_embed[b]).astype(NP_F8),
            "lw": lw_b.reshape(P, 8 * N_DC * NPC).astype(NP_F8),
        })
        counts.append((em.sum(), (1.0 - em).sum(), qm.sum(), (1.0 - qm).sum()))
    return in_maps, counts


def finalize(per_core, counts):
    """per_core: list of 8 arrays [2, 768] (epw|qw|qsqw) -> full (3,)."""
    pos = np.zeros(B)
    neg = np.zeros(B)
    for b in range(B):
        st = np.asarray(per_core[b]).astype(np.float64)
        n_pe, n_ne, n_pq, n_nq = counts[b]
        epw, qw, qsq = st[:, 0:C], st[:, C:2 * C], st[:, 2 * C:3 * C]
        pc = epw[0] / (n_pe + 0.1)
        ncen = epw[1] / (n_ne + 0.1)
        pn = qsq[0].sum() - 2.0 * (pc @ qw[0]) + n_pq * (pc @ pc)
        nn = qsq[1].sum() - 2.0 * (ncen @ qw[1]) + n_nq * (ncen @ ncen)
        pos[b] = pn / (max(n_pq, 1.0) * C) if n_pq > 0 else 0.0
        neg[b] = nn / (max(n_nq, 1.0) * C) if n_nq > 0 else 0.0
    return np.array(
        [(pos + neg).mean(), pos.mean(), neg.mean()], dtype=np.float32
    )


def kernel(ep_mask_embed, ep_mask, query_mask_embed, query_mask):
    ep_mask_embed = np.asarray(ep_mask_embed, dtype=np.float32)
    ep_mask = np.asarray(ep_mask, dtype=np.float32)
    query_mask_embed = np.asarray(query_mask_embed, dtype=np.float32)
    query_mask = np.asarray(query_mask, dtype=np.float32)

    nc = get_nc()
    in_maps, counts = make_in_maps(
        ep_mask_embed, ep_mask, query_mask_embed, query_mask)
    res = run_bass_kernel_spmd(nc, in_maps, list(range(B)))
    return finalize([r["out"] for r in res.results], counts)
